# revision 3
# baseline (speedup 1.0000x reference)
"""Trainium2 Bass kernel for AdaptiveDiffusionConv (gnn_message_passing).

Reference computation (per batch b):
    a   = adj * att[b]                      # [m, n]
    S   = [I, a, a @ a]
    rhs[k] = S[k]^T @ x[b]                  # [n, (f,t)]
    out = relu(sum_k rhs[k] @ Theta[k])     # [n, (o,t)]

Reformulated (a@a never materialized; Theta commutes with the node-dim matmul):
    out = relu( x@Th0 + a^T (x@Th1 + a^T (x@Th2)) )
where x@Thk is the f-contraction. The f-contraction is folded into the same
PSUM accumulation groups as the a^T matmuls by augmenting the contraction
dim with (f,t) rows: lhsT' = x^T[(f,t), n], rhs' = thblk[k] where
thblk[k][(f,t'), (o,t)] = Theta[k,f,o] * (t'==t)   (host-precomputed).

Sharding: pure data-parallel over batch B=16 across 8 cores (B_local=2).
adj / thblk / identity replicated; no collectives.
"""

import sys

sys.path.insert(0, "/opt/trn_rl_repo")

import numpy as np

import concourse.bacc as bacc
import concourse.mybir as mybir
from concourse import tile
from concourse.bass_utils import run_bass_kernel_spmd

B, N, F, T, K, O = 16, 1024, 16, 12, 3, 16
NCORES = 8
BL = B // NCORES  # 2 batches per core
P = 128
NT = N // P  # 8 node tiles
FT = F * T  # 192
OT = O * T  # 192
HC = FT // 2  # 96, contraction chunk for augmented rows

F32 = mybir.dt.float32
BF16 = mybir.dt.bfloat16

_CACHE = {}


def build_nc():
    nc = bacc.Bacc()

    x_ext = nc.declare_dram_parameter("x", [BL, N, F, T], F32, isOutput=False)
    att_ext = nc.declare_dram_parameter("att", [BL, N, N], F32, isOutput=False)
    adj_ext = nc.declare_dram_parameter("adj", [N, N], F32, isOutput=False)
    th_ext = nc.declare_dram_parameter("thblk", [K, 2, HC, OT], F32, isOutput=False)
    id_ext = nc.declare_dram_parameter("ident", [P, P], F32, isOutput=False)
    out_ext = nc.declare_dram_parameter("out", [BL, N, O, T], F32, isOutput=True)

    x_flat = x_ext.rearrange("b n f t -> b n (f t)")
    out_flat = out_ext.rearrange("b n o t -> b n (o t)")

    with tile.TileContext(nc) as tc:
        with (
            tc.tile_pool(name="const", bufs=1) as const,
            tc.tile_pool(name="thstg", bufs=2) as thstg,
            tc.tile_pool(name="attp", bufs=4) as attp,
            tc.tile_pool(name="xstg", bufs=4) as xstg,
            tc.tile_pool(name="big", bufs=1) as big,
            tc.tile_pool(name="tpp", bufs=2, space="PSUM") as tpp,
            tc.tile_pool(name="mmp", bufs=5, space="PSUM") as mmp,
            tc.tile_pool(name="resp", bufs=3) as resp,
        ):
            ident = const.tile([P, P], F32)
            nc.sync.dma_start(ident[:], id_ext[:])

            # striped Theta blocks, bf16: th[:, (k*2+c)*OT : ...]
            th = const.tile([HC, K * 2 * OT], BF16)
            for k in range(K):
                for c in range(2):
                    ts = thstg.tile([HC, OT], F32, tag="thst")
                    nc.sync.dma_start(ts[:], th_ext[k, c])
                    nc.vector.tensor_copy(
                        th[:, (k * 2 + c) * OT : (k * 2 + c) * OT + OT], ts[:]
                    )

            # persistent SBUF tensors
            adj_sb = big.tile([P, NT * N], F32)  # [128, 8*1024] f32
            a_sb = big.tile([P, BL * NT * N], BF16)  # [128, 16*1024] bf16
            xT = big.tile([HC, BL * 2 * N], BF16)  # [96, 4*1024] bf16
            vw = big.tile([P, BL * 2 * NT * OT], BF16)  # [128, 4*8*192] bf16

            def load_x(b):
                """x[b] -> xT[(f,t), n] bf16 via PE transpose."""
                for i in range(NT):
                    xs = xstg.tile([P, FT], F32, tag="xs")
                    nc.sync.dma_start(xs[:], x_flat[b, i * P : (i + 1) * P, :])
                    for c in range(2):
                        tp = tpp.tile([HC, P], F32, tag="tp")
                        nc.tensor.transpose(tp[:], xs[:, c * HC : (c + 1) * HC], ident[:])
                        nc.vector.tensor_copy(
                            xT[:, (b * 2 + c) * N + i * P : (b * 2 + c) * N + (i + 1) * P],
                            tp[:],
                        )

            def load_att(b):
                """a[b] = adj * att[b] -> bf16."""
                for j in range(NT):
                    at = attp.tile([P, N], F32, tag="att")
                    nc.sync.dma_start(at[:], att_ext[b, j * P : (j + 1) * P, :])
                    nc.vector.tensor_mul(
                        a_sb[:, (b * NT + j) * N : (b * NT + j) * N + N],
                        adj_sb[:, j * N : (j + 1) * N],
                        at[:],
                    )

            def stage(b, kk, rhs_base, writer):
                """psum[n_tile] = x@Th[kk] (+ a^T @ vw[rhs_base]) for each n tile."""
                for i in range(NT):
                    ps = mmp.tile([P, OT], F32, tag="mm")
                    last_aug = rhs_base is None
                    for c in range(2):
                        nc.tensor.matmul(
                            ps[:],
                            xT[:, (b * 2 + c) * N + i * P : (b * 2 + c) * N + (i + 1) * P],
                            th[:, (kk * 2 + c) * OT : (kk * 2 + c) * OT + OT],
                            start=(c == 0),
                            stop=(last_aug and c == 1),
                        )
                    if rhs_base is not None:
                        for j in range(NT):
                            nc.tensor.matmul(
                                ps[:],
                                a_sb[
                                    :,
                                    (b * NT + j) * N + i * P : (b * NT + j) * N + (i + 1) * P,
                                ],
                                vw[:, rhs_base + j * OT : rhs_base + (j + 1) * OT],
                                start=False,
                                stop=(j == NT - 1),
                            )
                    writer(i, ps)

            def copy_writer(dst_base):
                def w(i, ps):
                    nc.scalar.copy(vw[:, dst_base + i * OT : dst_base + (i + 1) * OT], ps[:])

                return w

            def out_writer(b):
                def w(i, ps):
                    res = resp.tile([P, OT], F32, tag="res")
                    nc.scalar.activation(
                        res[:], ps[:], mybir.ActivationFunctionType.Relu
                    )
                    nc.scalar.dma_start(out_flat[b, i * P : (i + 1) * P, :], res[:])

                return w

            def vbase(b, s):  # s=0: v2, s=1: w
                return (b * 2 + s) * NT * OT

            # ---- trace order (== scheduling priority / engine program order) ----
            load_x(0)  # small x DMAs first in sync FIFO
            load_x(1)
            for j in range(NT):  # adj load
                nc.sync.dma_start(
                    adj_sb[:, j * N : (j + 1) * N], adj_ext[j * P : (j + 1) * P, :]
                )
            load_att(0)

            # batch 0 compute
            stage(0, 2, None, copy_writer(vbase(0, 0)))  # v2 = x Th2
            stage(0, 1, vbase(0, 0), copy_writer(vbase(0, 1)))  # w = x Th1 + a^T v2
            load_att(1)  # b1 loads overlap b0 tail compute
            stage(0, 0, vbase(0, 1), out_writer(0))  # out0 = relu(x Th0 + a^T w)

            # batch 1 compute
            stage(1, 2, None, copy_writer(vbase(1, 0)))
            stage(1, 1, vbase(1, 0), copy_writer(vbase(1, 1)))
            stage(1, 0, vbase(1, 1), out_writer(1))

    nc.compile()
    return nc


def make_host_inputs(adj, Theta):
    thblk = np.zeros((K, FT, OT), np.float32)
    for t in range(T):
        rows = np.arange(F) * T + t
        cols = np.arange(O) * T + t
        for k in range(K):
            thblk[k][np.ix_(rows, cols)] = Theta[k]
    thblk = thblk.reshape(K, 2, HC, OT)
    ident = np.eye(P, dtype=np.float32)
    return {
        "adj": np.ascontiguousarray(adj, np.float32),
        "thblk": np.ascontiguousarray(thblk),
        "ident": ident,
    }


def kernel(x, spatial_attention, adj, Theta):
    x = np.asarray(x, np.float32)
    att = np.asarray(spatial_attention, np.float32)
    adj = np.asarray(adj, np.float32)
    Theta = np.asarray(Theta, np.float32)

    if "nc" not in _CACHE:
        _CACHE["nc"] = build_nc()
    nc = _CACHE["nc"]

    shared = make_host_inputs(adj, Theta)
    in_maps = []
    for c in range(NCORES):
        in_maps.append(
            {
                "x": np.ascontiguousarray(x[c * BL : (c + 1) * BL]),
                "att": np.ascontiguousarray(att[c * BL : (c + 1) * BL]),
                **shared,
            }
        )
    res = run_bass_kernel_spmd(nc, in_maps, core_ids=list(range(NCORES)))
    out = np.concatenate([res.results[c]["out"] for c in range(NCORES)], axis=0)
    return out.astype(np.float32)


# revision 5
# speedup vs baseline: 1.2749x; 1.2749x over previous
"""Trainium2 Bass kernel for AdaptiveDiffusionConv (gnn_message_passing).

Reference computation (per batch b):
    a   = adj * att[b]                      # [m, n]
    S   = [I, a, a @ a]
    rhs[k] = S[k]^T @ x[b]                  # [n, (f,t)]
    out = relu(sum_k rhs[k] @ Theta[k])     # [n, (o,t)]

Reformulated (a@a never materialized; Theta commutes with the node-dim matmul):
    out = relu( x@Th0 + a^T (x@Th1 + a^T (x@Th2)) )
where x@Thk is the f-contraction, folded into the same PSUM accumulation
groups as the a^T matmuls by augmenting the contraction dim with (f,t)
rows: lhsT' = x^T[(f,t), n], rhs' = thblk[k] where
thblk[k][(f,t'), (o,t)] = Theta[k,f,o] * (t'==t)   (host-precomputed).

Sharding: pure data-parallel over batch B=16 across 8 cores (B_local=2).
adj / thblk / identity replicated; no collectives.
"""

import sys

sys.path.insert(0, "/opt/trn_rl_repo")

import numpy as np

import concourse.bacc as bacc
import concourse.mybir as mybir
from concourse import tile
from concourse.bass_utils import run_bass_kernel_spmd

B, N, F, T, K, O = 16, 1024, 16, 12, 3, 16
NCORES = 8
BL = B // NCORES  # 2 batches per core
P = 128
NT = N // P  # 8 node tiles
FT = F * T  # 192
OT = O * T  # 192
HC = FT // 2  # 96, contraction chunk for augmented rows

F32 = mybir.dt.float32
BF16 = mybir.dt.bfloat16

_CACHE = {}


def build_nc():
    nc = bacc.Bacc()

    x_ext = nc.declare_dram_parameter("x", [BL, N, F, T], F32, isOutput=False)
    att_ext = nc.declare_dram_parameter("att", [BL, N, N], F32, isOutput=False)
    adj_ext = nc.declare_dram_parameter("adj", [N, N], F32, isOutput=False)
    th_ext = nc.declare_dram_parameter("thblk", [K, 2, HC, OT], F32, isOutput=False)
    id_ext = nc.declare_dram_parameter("ident", [P, P], F32, isOutput=False)
    out_ext = nc.declare_dram_parameter("out", [BL, N, O, T], F32, isOutput=True)

    x_tiled = x_ext.rearrange("b (i p) f t -> b p i (f t)", p=P)
    out_flat = out_ext.rearrange("b n o t -> b n (o t)")

    with tile.TileContext(nc) as tc:
        with (
            tc.tile_pool(name="const", bufs=1) as const,
            tc.tile_pool(name="thstg", bufs=2) as thstg,
            tc.tile_pool(name="attp", bufs=10) as attp,
            tc.tile_pool(name="big", bufs=1) as big,
            tc.tile_pool(name="psp", bufs=8, space="PSUM") as psp,
            tc.tile_pool(name="resp", bufs=8) as resp,
        ):
            ident = const.tile([P, P], F32)
            nc.gpsimd.dma_start(ident[:], id_ext[:])

            # striped Theta blocks, bf16: th[:, (k*2+c)*OT : ...]
            th = const.tile([HC, K * 2 * OT], BF16)
            for k in range(K):
                for c in range(2):
                    ts = thstg.tile([HC, OT], F32, tag="thst")
                    nc.gpsimd.dma_start(ts[:], th_ext[k, c])
                    nc.scalar.copy(
                        th[:, (k * 2 + c) * OT : (k * 2 + c) * OT + OT], ts[:]
                    )

            # persistent SBUF tensors
            adj_sb = big.tile([P, NT * N], F32)  # [128, 8*1024] f32
            a_sb = big.tile([P, BL * NT * N], BF16)  # [128, 16*1024] bf16
            xall = big.tile([P, BL * NT * FT], F32)  # [128, 3072] f32
            xT = big.tile([HC, BL * NT * 2 * P], BF16)  # [96, 4096] bf16
            vw = big.tile([P, BL * 2 * NT * OT], BF16)  # [128, 6144] bf16

            # ---- DMA issue order ----
            # sync queue: x (2 big), att0 (8), att1 (8)
            for b in range(BL):
                nc.sync.dma_start(
                    xall[:, b * NT * FT : (b + 1) * NT * FT], x_tiled[b]
                )
            att_tiles = {}
            for b in range(BL):
                for j in range(NT):
                    at = attp.tile([P, N], F32, tag="att")
                    nc.sync.dma_start(at[:], att_ext[b, j * P : (j + 1) * P, :])
                    att_tiles[(b, j)] = at
            # gpsimd queue: adj (8)  (ident/th already queued above)
            for j in range(NT):
                nc.gpsimd.dma_start(
                    adj_sb[:, j * N : (j + 1) * N], adj_ext[j * P : (j + 1) * P, :]
                )

            def a_slice(b, j, i):
                base = (b * NT + j) * N + i * P
                return a_sb[:, base : base + P]

            def xT_slice(b, i, c):
                base = ((b * NT + i) * 2 + c) * P
                return xT[:, base : base + P]

            def vw_slice(b, s, j):
                base = ((b * 2 + s) * NT + j) * OT
                return vw[:, base : base + OT]

            def th_slice(k, c):
                return th[:, (k * 2 + c) * OT : (k * 2 + c) * OT + OT]

            def mul_a(b, j):
                nc.vector.tensor_mul(
                    a_sb[:, (b * NT + j) * N : (b * NT + j) * N + N],
                    adj_sb[:, j * N : (j + 1) * N],
                    att_tiles[(b, j)][:],
                )

            def transpose_x(b):
                """x[b] tiles -> xT[(f,t), n] bf16, plus v2 seed handled separately."""
                for i in range(NT):
                    tp = psp.tile([HC, 2 * P], F32, tag="ps")
                    xs = xall[:, (b * NT + i) * FT : (b * NT + i) * FT + FT]
                    for c in range(2):
                        nc.tensor.transpose(
                            tp[:, c * P : (c + 1) * P], xs[:, c * HC : (c + 1) * HC], ident[:]
                        )
                    nc.vector.tensor_copy(
                        xT[:, (b * NT + i) * 2 * P : (b * NT + i + 1) * 2 * P], tp[:]
                    )

            def v2_stage(b):
                for i in range(NT):
                    ps = psp.tile([P, OT], F32, tag="ps")
                    for c in range(2):
                        nc.tensor.matmul(
                            ps[:],
                            xT_slice(b, i, c),
                            th_slice(2, c),
                            start=(c == 0),
                            stop=(c == 1),
                        )
                    nc.scalar.copy(vw_slice(b, 0, i)[:], ps[:])

            def w_stage(b):
                """j-ordered: psum groups for all 8 n-tiles open concurrently,
                a^T rank updates applied in m-tile arrival order."""
                pss = []
                for i in range(NT):
                    ps = psp.tile([P, OT], F32, tag="ps")
                    for c in range(2):
                        nc.tensor.matmul(
                            ps[:], xT_slice(b, i, c), th_slice(1, c),
                            start=(c == 0), stop=False,
                        )
                    pss.append(ps)
                for j in range(NT):
                    for i in range(NT):
                        nc.tensor.matmul(
                            pss[i][:], a_slice(b, j, i), vw_slice(b, 0, j),
                            start=False, stop=(j == NT - 1),
                        )
                for i in range(NT):
                    nc.scalar.copy(vw_slice(b, 1, i)[:], pss[i][:])

            def out_stage(b):
                for i in range(NT):
                    ps = psp.tile([P, OT], F32, tag="ps")
                    for c in range(2):
                        nc.tensor.matmul(
                            ps[:], xT_slice(b, i, c), th_slice(0, c),
                            start=(c == 0), stop=False,
                        )
                    for j in range(NT):
                        nc.tensor.matmul(
                            ps[:], a_slice(b, j, i), vw_slice(b, 1, j),
                            start=False, stop=(j == NT - 1),
                        )
                    res = resp.tile([P, OT], F32, tag="res")
                    nc.scalar.activation(
                        res[:], ps[:], mybir.ActivationFunctionType.Relu
                    )
                    nc.gpsimd.dma_start(out_flat[b, i * P : (i + 1) * P, :], res[:])

            # ---- compute trace order ----
            transpose_x(0)
            v2_stage(0)
            for j in range(NT):
                mul_a(0, j)
            w_stage(0)
            transpose_x(1)
            v2_stage(1)
            for j in range(NT):
                mul_a(1, j)
            out_stage(0)
            w_stage(1)
            out_stage(1)

    nc.compile()
    return nc


def make_host_inputs(adj, Theta):
    thblk = np.zeros((K, FT, OT), np.float32)
    for t in range(T):
        rows = np.arange(F) * T + t
        cols = np.arange(O) * T + t
        for k in range(K):
            thblk[k][np.ix_(rows, cols)] = Theta[k]
    thblk = thblk.reshape(K, 2, HC, OT)
    ident = np.eye(P, dtype=np.float32)
    return {
        "adj": np.ascontiguousarray(adj, np.float32),
        "thblk": np.ascontiguousarray(thblk),
        "ident": ident,
    }


def kernel(x, spatial_attention, adj, Theta):
    x = np.asarray(x, np.float32)
    att = np.asarray(spatial_attention, np.float32)
    adj = np.asarray(adj, np.float32)
    Theta = np.asarray(Theta, np.float32)

    if "nc" not in _CACHE:
        _CACHE["nc"] = build_nc()
    nc = _CACHE["nc"]

    shared = make_host_inputs(adj, Theta)
    in_maps = []
    for c in range(NCORES):
        in_maps.append(
            {
                "x": np.ascontiguousarray(x[c * BL : (c + 1) * BL]),
                "att": np.ascontiguousarray(att[c * BL : (c + 1) * BL]),
                **shared,
            }
        )
    res = run_bass_kernel_spmd(nc, in_maps, core_ids=list(range(NCORES)))
    out = np.concatenate([res.results[c]["out"] for c in range(NCORES)], axis=0)
    return out.astype(np.float32)
